# revision 2
# baseline (speedup 1.0000x reference)
"""ChebNet (K=5) forward on 8 Trainium2 NeuronCores — v2.

Design vs v1 baseline:
  - Host folds the full symmetric normalization into the selector weights
    (S = -dis[src]*w*dis[dst]); no on-device deg phase, no dis scaling.
    The Chebyshev 2x enters via the per-propagate fp16 staging scale.
  - Nodes re-assigned to cores balancing per-core in-edge counts (host
    permutation, inverted when assembling the output).
  - Edges flat-chunked within (supergroup, half) streams: chunks may span
    dst blocks (extra straddle matmuls) -> ~14% fewer gather indices.
  - AllGather output tile uses addr_space="Shared" (fast collective path).
  - T_k = Q - T_{k-2} computed directly PSUM -> SBUF on DVE (one op/block).
  - Gathers ride SWDGE queue `GQ` (num_swdge_queues=4 enabled).

Per propagate:
  hst = fp16(s * T)            (Scalar, s=1 for k=1 else 2)
  ag_in <- hst; hbuf = AllGather(ag_in)            (Shared out)
  per (sg, half) run: dma_gather gcalls (<=GCALL chunks) from hbuf window
  per chunk: S-matmuls into per-block PSUM (start/stop by op stream)
  per block: T_new = PSUM - T_prev (DVE)
  out_phase: oacc += transpose(T_k) @ W_k          (PE + DVE)
"""

import numpy as np

N = 50000
E = 600000
C = 128
K = 5
NC = 8
PB = 128
SLICE = 6272                  # nodes per core
NPAD = SLICE * NC             # 50176
NBLK = SLICE // PB            # 49
HALF = 32768
SGB = 6                       # dst blocks per supergroup (PSUM banks)
GCALL = 18                    # max chunks per dma_gather call
NSWQ = 4                      # SWDGE queues (gather descgen runs on the
                              # Q7 pair owning the queue -> 4x parallel)
QRR = (1, 2, 3, 0)            # round-robin queue order for gcalls

F16 = np.float16
F32 = np.float32


# ----------------------------------------------------------------------
# host-side plan
# ----------------------------------------------------------------------

def build_plan(edge_index, edge_weight):
    src = edge_index[0].astype(np.int64)
    dst = edge_index[1].astype(np.int64)
    w = edge_weight.astype(np.float64)

    # normalization (host): deg over src, dis = rsqrt
    deg = np.zeros(N, np.float64)
    np.add.at(deg, src, w)
    dis = np.where(deg > 0, 1.0 / np.sqrt(np.maximum(deg, 1e-20)), 0.0)
    norm_w = (-2.0 * dis[src] * w * dis[dst]).astype(F32)

    # node -> core assignment balancing in-degree (edge count per core)
    indeg = np.bincount(dst, minlength=N)
    order_nodes = np.argsort(-indeg, kind="stable")
    core_cnt = np.zeros(NC, np.int64)
    core_load = np.zeros(NC, np.float64)
    assign = np.zeros(NPAD, np.int64)  # node -> core
    # greedy: heaviest nodes first to least-loaded eligible core
    for nidx in order_nodes:
        c = np.argmin(np.where(core_cnt < SLICE, core_load, np.inf))
        assign[nidx] = c
        core_load[c] += indeg[nidx]
        core_cnt[c] += 1
    # pad nodes 50000..50175 fill remaining slots
    for nidx in range(N, NPAD):
        c = int(np.argmin(np.where(core_cnt < SLICE, core_cnt, np.inf)))
        assign[nidx] = c
        core_cnt[c] += 1
    # local index within core: stable by node id
    perm = np.zeros(NPAD, np.int64)      # node -> padded global position
    inv_positions = []
    posc = np.zeros(NC, np.int64)
    local_of = np.zeros(NPAD, np.int64)
    for nidx in range(NPAD):
        c = assign[nidx]
        local_of[nidx] = posc[c]
        perm[nidx] = c * SLICE + posc[c]
        posc[c] += 1
    assert np.all(posc == SLICE)

    # virtual (p-major) row of node n in hbuf: core r, local l = t*PB + p
    # -> m = r*SLICE + p*NBLK + t
    def vrow(n):
        c = assign[n]
        l = local_of[n]
        t, p = l // PB, l % PB
        return c * SLICE + p * NBLK + t

    msrc = np.array([0] * 0)
    # vectorized vrow
    c_of = assign
    l_of = local_of
    t_of = l_of // PB
    p_of = l_of % PB
    vrow_all = c_of * SLICE + p_of * NBLK + t_of

    msrc = vrow_all[src]
    dcore = assign[dst]
    dlocal = l_of[dst]
    dblk = dlocal // PB
    dpos = dlocal % PB
    half = (msrc >= HALF).astype(np.int64)

    n_sg = -(-NBLK // SGB)
    sgb_of_block = np.minimum(np.arange(NBLK) // SGB, n_sg - 1)
    sg_of_blk = sgb_of_block[dblk]          # per-edge supergroup

    # sort edges per core by (sg, half, blk, anything)
    key = (((dcore * n_sg + sg_of_blk) * 2 + half) * NBLK + dblk)
    order = np.argsort(key, kind="stable")

    # per (core, sg, half) edge counts
    cnt = np.zeros((NC, n_sg, 2), np.int64)
    for c in range(NC):
        m = dcore == c
        np.add.at(cnt[c], (sg_of_blk[m], half[m]), 1)

    # uniform chunks per (sg, half): max over cores
    nch_run = np.zeros((n_sg, 2), np.int64)
    for s in range(n_sg):
        for h in range(2):
            nch_run[s, h] = -(-cnt[:, s, h].max() // PB)

    # gcalls: split each run into <=GCALL chunk calls
    gcalls = []   # (sg, half, chunk0_in_run, nch, idx_col0)
    idx_cols = 0
    chunk_meta = []  # global chunk id -> (sg, half)
    for s in range(n_sg):
        for h in range(2):
            nch = int(nch_run[s, h])
            nsplit = max(1, -(-nch // GCALL))
            i = 0
            for si in range(nsplit):
                nn = (nch // nsplit) + (1 if si < nch % nsplit else 0)
                if nn == 0:
                    continue
                gcalls.append((s, h, i, nn, idx_cols))
                idx_cols += nn * 8
                i += nn
            for _ in range(nch):
                chunk_meta.append((s, h))
    TOTCH = len(chunk_meta)

    # per-core edge slots: [TOTCH, PB] of (vsrc_idx, weight, dblk, dpos)
    # build op stream: per chunk, list of (block, start, stop) + S tile
    idx_all = np.zeros((NC, 128, idx_cols), np.int16)
    # op stream is uniform across cores in count: ops = per chunk the set of
    # blocks present IN ANY core? No: S tiles per core differ but op COUNT
    # must be uniform. Use per-(chunk) fixed block windows:
    # chunks are flat within (sg,half); block composition differs per core!
    # -> make op stream per chunk = list of blocks covered, defined as the
    # UNION over cores? That can differ. Simpler: per chunk allow up to
    # MAXB blocks, padded with dummy ops into a scratch psum? Expensive.
    # Instead: fix slot->block mapping per (core) but op stream uniform by
    # taking for each (sg,half) run the per-core block boundaries rounded
    # to the max: we emit ops per (chunk, b) for every b whose [start,end)
    # chunk range (over cores' max extents) intersects the chunk.
    # Block b edges occupy chunk range [lo_b, hi_b) where lo/hi are in
    # units of chunks; per core they differ. Use the union range over
    # cores: ops = sum over b of (hi_b_max - lo_b_min) intersect chunks.
    ops_per_chunk = [[] for _ in range(TOTCH)]

    # compute per-core per-block slot ranges within runs
    # run chunk base:
    run_base = {}
    acc = 0
    for s in range(n_sg):
        for h in range(2):
            run_base[(s, h)] = acc
            acc += int(nch_run[s, h])

    # per core: for each (sg,half) run, the edges sorted; block boundaries
    ecnt_blk = np.zeros((NC, NBLK, 2), np.int64)
    for c in range(NC):
        m = dcore == c
        np.add.at(ecnt_blk[c], (dblk[m], half[m]), 1)

    # block chunk-extent per run (over all cores)
    for s in range(n_sg):
        blks = [b for b in range(NBLK) if sgb_of_block[b] == s]
        for h in range(2):
            base = run_base[(s, h)]
            nch = int(nch_run[s, h])
            for c in range(NC):
                off = 0
                for b in blks:
                    nb = int(ecnt_blk[c, b, h])
                    if nb == 0:
                        continue
                    lo_ch = off // PB
                    hi_ch = (off + nb - 1) // PB
                    for ch in range(lo_ch, hi_ch + 1):
                        lst = ops_per_chunk[base + ch]
                        if b not in lst:
                            lst.append(b)
                    off += nb
    for lst in ops_per_chunk:
        lst.sort()

    n_ops = sum(len(l) for l in ops_per_chunk)

    # op -> sv column layout: ops packed in stream order, one 128x128 tile
    # per op. Slabs = per gcall: all ops of its chunks.
    op_stream = []  # (chunk, block, sv_slab, sv_off)
    gcall_ops = []  # per gcall: list of op indices
    slab_widths = []
    for gi, (s, h, ch0, nn, col0) in enumerate(gcalls):
        base = run_base[(s, h)]
        ops = []
        off = 0
        for ch in range(ch0, ch0 + nn):
            for b in ops_per_chunk[base + ch]:
                ops.append((base + ch, b, gi, off))
                off += 1
        gcall_ops.append(list(range(len(op_stream), len(op_stream) + len(ops))))
        op_stream.extend(ops)
        slab_widths.append(off)
    MAXW = max(slab_widths)

    # build S data + gather idx per core
    sv_all = np.zeros((NC, len(gcalls), PB, MAXW * PB), F16)
    for c in range(NC):
        eids = order[dcore[order] == c]
        # slot layout: per (sg, half) run, edges packed flat; chunk ch slot p
        # -> edge index eids[pos] with pos = (run offset) + ch*PB + p
        run_off = {}
        acc2 = 0
        for s in range(n_sg):
            for h in range(2):
                run_off[(s, h)] = acc2
                acc2 += int(cnt[c, s, h])
        # per run: slots [nch*PB], edges fill first cnt slots
        # gather idx:
        for gi, (s, h, ch0, nn, col0) in enumerate(gcalls):
            ncnt = int(cnt[c, s, h])
            base_e = run_off[(s, h)]
            sl_idx = np.zeros(nn * PB, np.int64)
            for j in range(nn * PB):
                pos = (ch0 * PB) + j
                if pos < ncnt:
                    e = eids[base_e + pos]
                    sl_idx[j] = msrc[e] - h * HALF
                else:
                    sl_idx[j] = 0
            colb = sl_idx.reshape(-1, 16).T.astype(np.int16)  # [16, nn*8]
            idx_all[c, :, col0:col0 + nn * 8] = np.tile(colb, (8, 1))
        # S tiles
        for gi, (s, h, ch0, nn, col0) in enumerate(gcalls):
            ncnt = int(cnt[c, s, h])
            base_e = run_off[(s, h)]
            base_ch = run_base[(s, h)]
            for oi in gcall_ops[gi]:
                ch, b, _gi, off = op_stream[oi]
                ch_local = ch - base_ch
                t = np.zeros((PB, PB), F32)
                for p in range(PB):
                    pos = (ch_local * PB) + p
                    if pos >= ncnt:
                        continue
                    e = eids[base_e + pos]
                    if dblk[e] == b:
                        t[p, dpos[e]] = norm_w[e]
                sv_all[c, gi, :, off * PB:(off + 1) * PB] = t.astype(F16)

    sched = dict(gcalls=gcalls, gcall_ops=gcall_ops, op_stream=op_stream,
                 ops_per_chunk=ops_per_chunk, run_base=run_base,
                 nch_run=nch_run, idx_cols=idx_cols, TOTCH=TOTCH,
                 MAXW=MAXW, n_sg=n_sg, slab_widths=slab_widths,
                 n_ops=n_ops)
    return sched, idx_all, sv_all, perm


# ----------------------------------------------------------------------
# device program
# ----------------------------------------------------------------------

def build_program(sched):
    import concourse.bass as bass
    import concourse.bacc as bacc
    import concourse.mybir as mybir
    import concourse.tile as tile

    dt = mybir.dt
    AF = mybir.ActivationFunctionType
    ALU = mybir.AluOpType
    RG = [list(range(NC))]

    gcalls = sched["gcalls"]
    gcall_ops = sched["gcall_ops"]
    op_stream = sched["op_stream"]
    idx_cols = sched["idx_cols"]
    MAXW = sched["MAXW"]

    nc = bacc.Bacc("TRN2", target_bir_lowering=False, debug=False,
                   num_devices=NC, num_swdge_queues=NSWQ)

    xf_h = nc.dram_tensor("x_pm16", [128, NBLK * C], dt.float16, kind="ExternalInput")
    x2_h = nc.dram_tensor("x2_pm16", [128, NBLK * C], dt.float16, kind="ExternalInput")
    idx_h = nc.dram_tensor("idx", [128, idx_cols], dt.int16, kind="ExternalInput")
    sv_h = nc.dram_tensor("sv", [len(gcalls), PB, MAXW * PB], dt.float16,
                          kind="ExternalInput")
    w_h = nc.dram_tensor("wmat", [C, K * C], dt.float16, kind="ExternalInput")
    bias_h = nc.dram_tensor("biasb", [128, C], dt.float32, kind="ExternalInput")
    id_h = nc.dram_tensor("ident", [128, 128], dt.float16, kind="ExternalInput")
    out_h = nc.dram_tensor("out_sl", [SLICE, C], dt.float32, kind="ExternalOutput")

    with tile.TileContext(nc) as tc:
        import contextlib
        ctx = contextlib.ExitStack()
        with ctx:
            const = ctx.enter_context(tc.tile_pool(name="const", bufs=1))
            big = ctx.enter_context(tc.tile_pool(name="big", bufs=1))
            dram = ctx.enter_context(tc.tile_pool(name="dram", bufs=2, space="DRAM"))

            idx_sb = const.tile([128, idx_cols], dt.int16)
            nc.sync.dma_start(idx_sb[:], idx_h[:, :])
            w_sb = const.tile([C, K * C], dt.float16)
            nc.sync.dma_start(w_sb[:], w_h[:, :])
            bias_sb = const.tile([128, C], dt.float32)
            nc.sync.dma_start(bias_sb[:], bias_h[:, :])
            id_sb = const.tile([128, 128], dt.float16)
            nc.sync.dma_start(id_sb[:], id_h[:, :])

            slots = [big.tile([128, NBLK * C], dt.float16, tag=f"slot{i}",
                              name=f"slot{i}") for i in range(3)]
            oacc = big.tile([128, NBLK * C], dt.float32)

            gp = ctx.enter_context(tc.tile_pool(name="gp", bufs=12))
            sp = ctx.enter_context(tc.tile_pool(name="sp", bufs=3))
            hstg = ctx.enter_context(tc.tile_pool(name="hstg", bufs=1))
            tstg = ctx.enter_context(tc.tile_pool(name="tstg", bufs=3))
            ostg = ctx.enter_context(tc.tile_pool(name="ostg", bufs=3))
            qps = ctx.enter_context(tc.tile_pool(name="qps", bufs=SGB, space="PSUM"))
            tps = ctx.enter_context(tc.tile_pool(name="tps", bufs=1, space="PSUM"))
            ops_ = ctx.enter_context(tc.tile_pool(name="ops", bufs=1, space="PSUM"))

            def band(t, b):
                return t[:, b * C:(b + 1) * C]

            def stage_and_ag(tsrc, fp16_src=None):
                """AllGather the fp16 S slot -> Shared hbuf."""
                ag_in = dram.tile([128, NBLK * C], dt.float16, tag="agin",
                                  name="ag_in")
                if fp16_src is not None:
                    nc.sync.dma_start(ag_in[:, :], fp16_src[:, :])
                else:
                    nc.sync.dma_start(ag_in[:, :], tsrc[:, :])
                hbuf = dram.tile([NC * 128, NBLK * C], dt.float16, tag="hbuf",
                                 name="hbuf", addr_space="Shared")
                nc.gpsimd.collective_compute(
                    "AllGather", mybir.AluOpType.bypass, replica_groups=RG,
                    ins=[ag_in.opt()], outs=[hbuf.opt()])
                return hbuf.rearrange("a (t c) -> (a t) c", c=C)

            NSEM = 12    # == gp bufs: same-sem preps are >= bufs apart,
                         # serialized by the gt-buffer WAW -> no cumulative
                         # sem-count races across rings
            gsems = [nc.alloc_semaphore(f"gdma{i}") for i in range(NSEM)]

            def propagate_into(hbuf, dst_slot, prev_slot, first_k,
                               filler=None):
                """Q = gathers + S matmuls; dst = Q - prev (or Q if first).

                Gathers are issued as prepare_only descgen (grouped to the
                gt buffer depth) + per-queue trigger_dma; descriptor
                generation runs ahead of (and across) the AllGather.
                `filler` is a list of closures (PE/DVE work for the
                previous k) interleaved after each group's matmuls.
                """
                q_open = {}
                done = {}
                need = {}
                for _oi, (_ch, _b, _gi, _off) in enumerate(op_stream):
                    need[_b] = need.get(_b, 0) + 1
                GRP = 12     # == gp bufs
                ngrp = -(-len(gcalls) // GRP)
                filler = list(filler or [])
                nfill = -(-len(filler) // max(1, ngrp))
                for g0 in range(0, len(gcalls), GRP):
                    grp = list(range(g0, min(g0 + GRP, len(gcalls))))
                    gts = {}
                    used_q = []
                    for gi in grp:
                        s, h, ch0, nn, col0 = gcalls[gi]
                        gt = gp.tile([128, GCALL, C], dt.float16, tag="g",
                                     name="gt")
                        gts[gi] = gt
                        src_ap = (hbuf[:HALF, :] if h == 0
                                  else hbuf[HALF:NPAD, :])
                        q = QRR[gi % len(QRR)]
                        if q not in used_q:
                            used_q.append(q)
                        nc.gpsimd.dma_gather(
                            gt[:, :nn, :], src_ap,
                            idx_sb[:, col0:col0 + nn * 8],
                            num_idxs=nn * 128, num_idxs_reg=nn * 128,
                            elem_size=C, single_packet=False,
                            queue_num=q)
                    for gi in grp:
                        s, h, ch0, nn, col0 = gcalls[gi]
                        gt = gts[gi]
                        st = sp.tile([128, MAXW * PB], dt.float16, tag="s",
                                     name="svt")
                        wgi = sched["slab_widths"][gi]
                        nc.scalar.dma_start(st[:, :wgi * PB],
                                            sv_h[gi, :, :wgi * PB])
                        base_ch = sched["run_base"][(s, h)]
                        for oi in gcall_ops[gi]:
                            ch, b, _gi, off = op_stream[oi]
                            j = ch - base_ch - ch0
                            if b not in q_open:
                                q_open[b] = qps.tile([128, C], dt.float32,
                                                     tag="q", name="q")
                            ps = q_open[b]
                            nd = done.get(b, 0)
                            last = nd + 1 == need[b]
                            nc.tensor.matmul(
                                ps[:], st[:, off * PB:(off + 1) * PB],
                                gt[:, j, :], start=(nd == 0), stop=last)
                            done[b] = nd + 1
                            if last:
                                if first_k:
                                    nc.vector.tensor_copy(band(dst_slot, b),
                                                          ps[:])
                                else:
                                    q16 = tstg.tile([128, C], dt.float16,
                                                    tag="q16", name="q16")
                                    nc.vector.tensor_copy(q16[:], ps[:])
                                    nc.vector.tensor_tensor(
                                        band(dst_slot, b), q16[:],
                                        band(prev_slot, b), ALU.subtract)
                                del q_open[b]
                    for _ in range(nfill):
                        if filler:
                            filler.pop(0)()
                for f in filler:
                    f()

            def out_band(tsrc, k, b, first):
                tp = tps.tile([128, 128], dt.float16, tag="tp", name="tp")
                nc.tensor.transpose(tp[:], band(tsrc, b), id_sb[:])
                tt = tstg.tile([128, 128], dt.float16, tag="tt", name="tt")
                nc.vector.tensor_copy(tt[:], tp[:])
                op = ops_.tile([128, C], dt.float32, tag="op", name="op")
                nc.tensor.matmul(op[:], tt[:], w_sb[:, k * C:(k + 1) * C],
                                 start=True, stop=True)
                if first:
                    nc.scalar.copy(band(oacc, b), op[:])
                else:
                    nc.vector.tensor_tensor(band(oacc, b), band(oacc, b),
                                            op[:], ALU.add)

            def out_fillers(tsrc, k, first):
                return [(lambda b=b: out_band(tsrc, k, b, first))
                        for b in range(NBLK)]

            # S0 = 2x (fp16, host-provided)
            nc.sync.dma_start(slots[0][:, :], x2_h[:, :])
            # k=0: AG of fp16 x (undoubled) directly from host tensor
            hbuf = stage_and_ag(None, fp16_src=xf_h)

            # k=1: S1 = PSUM (selectors carry the 2x); k=0 out_phase rides
            # as filler under the gather descgen
            propagate_into(hbuf, slots[1], None, True,
                           filler=out_fillers(slots[0], 0, True))
            hbuf = stage_and_ag(slots[1])

            # k=2..4: S_k = PSUM - S_{k-2}
            cur, prev = 1, 0
            for k in range(2, K):
                nxt = 3 - cur - prev
                propagate_into(hbuf, slots[nxt], slots[prev], False,
                               filler=out_fillers(slots[cur], k - 1, False))
                if k < K - 1:
                    hbuf = stage_and_ag(slots[nxt])
                prev, cur = cur, nxt

            # final out_phase for T4 (no propagate to hide it under)
            for f in out_fillers(slots[cur], K - 1, False):
                f()

            # final: out = relu(oacc + bias)
            for b in range(NBLK):
                fs = ostg.tile([128, C], dt.float32, tag="fs", name="fs")
                nc.vector.tensor_tensor(fs[:], band(oacc, b), bias_sb[:],
                                        ALU.add)
                nc.scalar.activation(fs[:], fs[:], AF.Relu)
                nc.sync.dma_start(out_h[b * PB:(b + 1) * PB, :], fs[:])

    nc.compile()
    return nc


# ----------------------------------------------------------------------
# entry point
# ----------------------------------------------------------------------

def make_in_maps(inputs, sched, idx_all, sv_all, perm):
    x = np.asarray(inputs["x"], F32)
    lw = np.asarray(inputs["lins_w"], F32)
    bias = np.asarray(inputs["bias"], F32)
    xp = np.zeros((NPAD, C), F32)
    xp[perm[:N]] = x           # permuted layout
    wmat = np.concatenate([lw[k] * 0.5 for k in range(K)], axis=1).astype(F32)
    biasb = np.tile(bias[None, :], (128, 1)).astype(F32)
    ident = np.eye(128, dtype=F16)
    in_maps = []
    for c in range(NC):
        xs = xp[c * SLICE:(c + 1) * SLICE]
        x_pm = np.ascontiguousarray(
            xs.reshape(NBLK, PB, C).transpose(1, 0, 2).reshape(128, NBLK * C))
        in_maps.append({
            "x_pm16": x_pm.astype(F16),
            "x2_pm16": (2.0 * x_pm).astype(F16),
            "idx": np.ascontiguousarray(idx_all[c]),
            "sv": np.ascontiguousarray(sv_all[c]),
            "wmat": wmat.astype(F16),
            "biasb": biasb,
            "ident": ident,
        })
    return in_maps


def _run(inputs, trace=False):
    from concourse import bass_utils

    ei = np.asarray(inputs["edge_index"])
    ew = np.asarray(inputs["edge_weight"], F32)
    sched, idx_all, sv_all, perm = build_plan(ei, ew)
    nc = build_program(sched)
    in_maps = make_in_maps(inputs, sched, idx_all, sv_all, perm)

    res = bass_utils.run_bass_kernel_spmd(
        nc, in_maps, core_ids=list(range(NC)), trace=trace)
    full = np.concatenate([res.results[c]["out_sl"] for c in range(NC)],
                          axis=0)
    out = full[perm[:N]].astype(F32)
    return out, res


def kernel(**inputs):
    out, _ = _run(inputs, trace=False)
    return out
